# revision 21
# baseline (speedup 1.0000x reference)
"""Multi-head self-attention (N=2, S=2048, E=1024, 16 heads) on 8 trn2 cores.

Sharding: data parallel over batch (2) x tensor parallel over heads (4 groups
of 4 heads). Each core computes in_proj for its local heads, attention with
full SxS scores for its local heads, and a partial out_proj (contraction over
its local 256 features). Host sums the 4 partials per batch and adds b_o.

Device kernel (per core), all matmuls in float32r (TF32-like, full PE rate):
  phase 1: qT/kT = W_{q,k} @ x^T  (features on partitions), V natural layout
           with a ones-column appended per head (softmax denominators).
  phase 2: scores transposed sT[k, q] = K Q^T per 128-k tile; exp on ACT;
           attnV out^T[d, q] accumulated over k, the ones row yielding
           sum_k exp; divide via reciprocal + gpsimd partition_broadcast +
           DVE multiply; out_proj fused per query block.
"""
import numpy as np

import concourse.bacc as bacc
import concourse.mybir as mybir
from concourse.tile import TileContext
from concourse.bass import ts

F32 = mybir.dt.float32
F32R = mybir.dt.float32r
EXP = mybir.ActivationFunctionType.Exp

D_MODEL = 1024
NHEAD = 16
DH = 64
N_BATCH = 2
SEQ = 2048
N_CORES = 8
GROUPS = 4            # head groups (cores per batch)
HL = NHEAD // GROUPS  # local heads per core = 4
FL = HL * DH          # local feature width = 256


def build_mha(nc, S=SEQ, E=D_MODEL, EOUT=D_MODEL, HLOC=HL, scale=0.125):
    """Emit the per-core kernel IR. Returns nothing; declares DRAM I/O."""
    FLOC = HLOC * DH          # local q/k/v feature count
    EC = E // 128             # contraction chunks for in_proj
    FT = FLOC // 128          # feature tiles for qT/kT (heads per tile = 2)
    TT = S // 128             # token tiles
    QB = S // 512             # 512-wide query blocks
    KT = S // 128             # 128-wide key tiles
    OC = FLOC // 128          # out_proj contraction chunks
    EB = (EOUT + 511) // 512  # out_proj output blocks
    TPQ = TT // QB            # token tiles per query block (4)

    xT = nc.dram_tensor("xT", [E, S], F32R, kind="ExternalInput")
    wT = nc.dram_tensor("wT", [E, 3 * FLOC], F32R, kind="ExternalInput")
    qkb = nc.dram_tensor("qkb", [128, 2 * FT], F32, kind="ExternalInput")
    vbr = nc.dram_tensor("vbr", [128, FLOC], F32, kind="ExternalInput")
    woT = nc.dram_tensor("woT", [FLOC, EOUT], F32R, kind="ExternalInput")
    vones = nc.dram_tensor("vones", [128, TT * HLOC], F32R, kind="ExternalInput")
    out = nc.dram_tensor("out", [S, EOUT], F32, kind="ExternalOutput")

    with TileContext(nc) as tc:
        with tc.tile_pool(name="persist", bufs=1) as pp:
            qkb_sb = pp.tile([128, 2 * FT], F32)
            nc.sync.dma_start(qkb_sb[:], qkb[:])
            vbr_sb = pp.tile([128, FLOC], F32)
            nc.sync.dma_start(vbr_sb[:], vbr[:])
            qT = pp.tile([128, FT, S], F32R)
            kT = pp.tile([128, FT, S], F32R)
            v = pp.tile([128, TT, HLOC, 65], F32R)
            outT = pp.tile([128, OC, S], F32R)
            woT_sb = pp.tile([128, OC, EOUT], F32R)

            # ---- phase 1: in_proj ----
            with tc.tile_pool(name="ph1", bufs=1) as p1, \
                 tc.tile_pool(name="ph1ps", bufs=2, space="PSUM") as ps1:
                xT_sb = p1.tile([128, EC, S], F32R)
                wT_sb = p1.tile([128, EC, 3 * FLOC], F32R)
                for c in range(EC):
                    nc.sync.dma_start(wT_sb[:, c, 0:2 * FLOC],
                                      wT[ts(c, 128), 0:2 * FLOC])
                    nc.sync.dma_start(xT_sb[:, c, 0:512], xT[ts(c, 128), 0:512])
                for tb in range(1, S // 512):
                    for c in range(EC):
                        nc.sync.dma_start(xT_sb[:, c, ts(tb, 512)],
                                          xT[ts(c, 128), ts(tb, 512)])
                for c in range(EC):
                    nc.sync.dma_start(wT_sb[:, c, 2 * FLOC:],
                                      wT[ts(c, 128), 2 * FLOC:])
                for c in range(OC):
                    nc.sync.dma_start(woT_sb[:, c, :], woT[ts(c, 128), :])
                nc.sync.dma_start(
                    v[:, :, :, 64:65],
                    vones.rearrange("p (t h one) -> p t h one", h=HLOC, one=1))

                # q/k first so kT/qT complete early and attention can begin
                # while the V projection still runs
                for tb in range(S // 512):
                    for ft in range(FT):
                        for gi, (dst, off) in enumerate(((qT, 0), (kT, FLOC))):
                            pq = ps1.tile([128, 512], F32, tag="pq")
                            lo = off + ft * 128
                            for c in range(EC):
                                nc.tensor.matmul(
                                    pq[:], wT_sb[:, c, lo:lo + 128],
                                    xT_sb[:, c, ts(tb, 512)],
                                    start=(c == 0), stop=(c == EC - 1))
                            nc.vector.tensor_scalar_add(
                                dst[:, ft, ts(tb, 512)], pq[:],
                                qkb_sb[:, gi * FT + ft:gi * FT + ft + 1])
                # V natural layout: [tok, vfeat] per 128-token tile
                for t in range(TT):
                    pv = ps1.tile([128, FLOC], F32, tag="pv")
                    for c in range(EC):
                        nc.tensor.matmul(
                            pv[:], xT_sb[:, c, ts(t, 128)],
                            wT_sb[:, c, 2 * FLOC:3 * FLOC],
                            start=(c == 0), stop=(c == EC - 1))
                    nc.vector.tensor_add(
                        v[:, t, :, 0:64],
                        pv.rearrange("p (h d) -> p h d", h=HLOC),
                        vbr_sb.rearrange("p (h d) -> p h d", h=HLOC))

            # ---- phase 2+3: attention with fused out_proj per query block ----
            with tc.tile_pool(name="ph2", bufs=16) as p2, \
                 tc.tile_pool(name="ph2oc", bufs=2) as p2oc, \
                 tc.tile_pool(name="ph3", bufs=2) as p3, \
                 tc.tile_pool(name="ph2ps", bufs=2, space="PSUM") as ps2, \
                 tc.tile_pool(name="ph2po", bufs=1, space="PSUM") as ps2o, \
                 tc.tile_pool(name="ph3ps", bufs=1, space="PSUM") as ps3:
                onum = 2
                for qb in range(QB):
                    for hp in range(HLOC // 2):
                        o0 = ps2o.tile([65, 512], F32, tag=f"o{onum % 3}")
                        o1 = ps2o.tile([65, 512], F32, tag=f"o{(onum + 1) % 3}")
                        onum += 2
                        oo = [o0, o1]
                        for kt in range(KT):
                            sps = ps2.tile([128, 2, 512], F32, tag="s")
                            ex = p2.tile([128, 2, 512], F32R, tag="exp")
                            for hh in range(2):
                                p0 = 64 * hh
                                nc.tensor.matmul(
                                    sps[:, hh, :],
                                    kT[p0:p0 + 64, hp, ts(kt, 128)],
                                    qT[p0:p0 + 64, hp, ts(qb, 512)],
                                    start=True, stop=True)
                            nc.scalar.activation(ex[:], sps[:], EXP, scale=scale)
                            for hh in range(2):
                                nc.tensor.matmul(
                                    oo[hh][:],
                                    v[:, kt, 2 * hp + hh, :],
                                    ex[:, hh, :],
                                    start=(kt == 0),
                                    stop=(kt == KT - 1))
                        for hh in range(2):
                            # copy out of PSUM promptly so the o slot frees for
                            # the next head pair; divide from the SBUF copy
                            oc = p2oc.tile([65, 512], F32, tag="oc")
                            nc.vector.tensor_copy(oc[:], oo[hh][:])
                            rec = p2oc.tile([1, 512], F32, tag="rec")
                            nc.vector.reciprocal(rec[:], oc[64:65, :])
                            rep = p2oc.tile([64, 512], F32, tag="rep")
                            nc.gpsimd.partition_broadcast(rep[:], rec[:])
                            # outT chunk hp holds feats of heads (2hp, 2hp+1)
                            nc.vector.tensor_mul(
                                outT[64 * hh:64 * hh + 64, hp, ts(qb, 512)],
                                oc[0:64, :], rep[:])
                    # out_proj for this query block's token tiles
                    for t in range(TPQ * qb, TPQ * qb + TPQ):
                        fo = p3.tile([128, EOUT], F32, tag="fo")
                        for eb in range(EB):
                            w = min(512, EOUT - eb * 512)
                            po = ps3.tile([128, 512], F32, tag="po")
                            for c in range(OC):
                                nc.tensor.matmul(
                                    po[:, :w], outT[:, c, ts(t, 128)],
                                    woT_sb[:, c, eb * 512:eb * 512 + w],
                                    start=(c == 0), stop=(c == OC - 1))
                            nc.vector.tensor_copy(fo[:, eb * 512:eb * 512 + w],
                                                  po[:, :w])
                        nc.sync.dma_start(out[ts(t, 128), :], fo[:])


_CACHED = {}


def _get_module():
    if "nc" not in _CACHED:
        nc = bacc.Bacc("TRN2")
        build_mha(nc)
        nc.finalize()
        _CACHED["nc"] = nc
    return _CACHED["nc"]


def make_in_maps(query, w_in, b_in, w_o):
    """Host-side sharding: per-core input dicts (layout transforms included)."""
    E, HLoc, FLoc = D_MODEL, HL, FL
    woT_full = np.ascontiguousarray(w_o.T, dtype=np.float32)  # (e_in, e_out)
    vones_arr = np.ones((128, (SEQ // 128) * HLoc), np.float32)
    in_maps = []
    for core in range(N_CORES):
        b, g = divmod(core, GROUPS)
        rows = np.r_[g * FLoc:(g + 1) * FLoc,
                     E + g * FLoc:E + (g + 1) * FLoc,
                     2 * E + g * FLoc:2 * E + (g + 1) * FLoc]
        bl = b_in[rows].astype(np.float32)
        ft_n = FLoc // 128
        qkb_c = np.empty((128, 2 * ft_n), np.float32)
        for ft in range(ft_n):
            qkb_c[:, ft] = bl[ft * 128:(ft + 1) * 128]
            qkb_c[:, ft_n + ft] = bl[FLoc + ft * 128:FLoc + (ft + 1) * 128]
        vbr_c = np.ascontiguousarray(
            np.broadcast_to(bl[2 * FLoc:], (128, FLoc)))
        in_maps.append({
            "vones": vones_arr,
            "xT": np.ascontiguousarray(query[b].T, dtype=np.float32),
            "wT": np.ascontiguousarray(w_in[rows].T, dtype=np.float32),
            "qkb": qkb_c, "vbr": vbr_c,
            "woT": np.ascontiguousarray(woT_full[g * FLoc:(g + 1) * FLoc]),
        })
    return in_maps


def kernel(query, key, value, w_in, b_in, w_o, b_o, _trace=False):
    from concourse.bass_utils import run_bass_kernel_spmd
    query = np.asarray(query, dtype=np.float32)
    nc = _get_module()
    in_maps = make_in_maps(query, np.asarray(w_in), np.asarray(b_in),
                           np.asarray(w_o))
    res = run_bass_kernel_spmd(nc, in_maps, core_ids=list(range(N_CORES)),
                               trace=_trace)
    out = np.empty((N_BATCH, SEQ, D_MODEL), np.float32)
    for b in range(N_BATCH):
        acc = res.results[b * GROUPS]["out"].astype(np.float32)
        for g in range(1, GROUPS):
            acc = acc + res.results[b * GROUPS + g]["out"]
        out[b] = acc + np.asarray(b_o, dtype=np.float32)[None, :]
    if _trace:
        kernel.last_exec_time_ns = res.exec_time_ns
    return out


# revision 23
# speedup vs baseline: 1.0091x; 1.0091x over previous
"""Multi-head self-attention (N=2, S=2048, E=1024, 16 heads) on 8 trn2 cores.

Sharding: data parallel over batch (2) x tensor parallel over heads (4 groups
of 4 heads). Each core computes in_proj for its local heads, attention with
full SxS scores for its local heads, and a partial out_proj (contraction over
its local 256 features). Host sums the 4 partials per batch and adds b_o.

Device kernel (per core), all matmuls in float32r (TF32-like, full PE rate):
  phase 1: qT/kT = W_{q,k} @ x^T  (features on partitions), V natural layout
           with a ones-column appended per head (softmax denominators).
  phase 2: scores transposed sT[k, q] = K Q^T per 128-k tile; exp on ACT;
           attnV out^T[d, q] accumulated over k, the ones row yielding
           sum_k exp; divide via reciprocal + gpsimd partition_broadcast +
           DVE multiply; out_proj fused per query block.
"""
import numpy as np

import concourse.bacc as bacc
import concourse.mybir as mybir
from concourse.tile import TileContext
from concourse.bass import ts

F32 = mybir.dt.float32
F32R = mybir.dt.float32r
EXP = mybir.ActivationFunctionType.Exp

D_MODEL = 1024
NHEAD = 16
DH = 64
N_BATCH = 2
SEQ = 2048
N_CORES = 8
GROUPS = 4            # head groups (cores per batch)
HL = NHEAD // GROUPS  # local heads per core = 4
FL = HL * DH          # local feature width = 256


def build_mha(nc, S=SEQ, E=D_MODEL, EOUT=D_MODEL, HLOC=HL, scale=0.125):
    """Emit the per-core kernel IR. Returns nothing; declares DRAM I/O."""
    FLOC = HLOC * DH          # local q/k/v feature count
    EC = E // 128             # contraction chunks for in_proj
    FT = FLOC // 128          # feature tiles for qT/kT (heads per tile = 2)
    TT = S // 128             # token tiles
    QB = S // 512             # 512-wide query blocks
    KT = S // 128             # 128-wide key tiles
    OC = FLOC // 128          # out_proj contraction chunks
    EB = (EOUT + 511) // 512  # out_proj output blocks
    TPQ = TT // QB            # token tiles per query block (4)

    xT = nc.dram_tensor("xT", [E, S], F32R, kind="ExternalInput")
    wT = nc.dram_tensor("wT", [E, 3 * FLOC], F32R, kind="ExternalInput")
    qkb = nc.dram_tensor("qkb", [128, 2 * FT], F32, kind="ExternalInput")
    vbr = nc.dram_tensor("vbr", [128, FLOC], F32, kind="ExternalInput")
    woT = nc.dram_tensor("woT", [FLOC, EOUT], F32R, kind="ExternalInput")
    vones = nc.dram_tensor("vones", [128, TT * HLOC], F32R, kind="ExternalInput")
    out = nc.dram_tensor("out", [S, EOUT], F32, kind="ExternalOutput")

    with TileContext(nc) as tc:
        with tc.tile_pool(name="persist", bufs=1) as pp:
            qkb_sb = pp.tile([128, 2 * FT], F32)
            nc.sync.dma_start(qkb_sb[:], qkb[:])
            vbr_sb = pp.tile([128, FLOC], F32)
            nc.sync.dma_start(vbr_sb[:], vbr[:])
            qT = pp.tile([128, FT, S], F32R)
            kT = pp.tile([128, FT, S], F32R)
            v = pp.tile([128, TT, HLOC, 65], F32R)
            outT = pp.tile([128, OC, S], F32R)
            woT_sb = pp.tile([128, OC, EOUT], F32R)

            # ---- phase 1: in_proj ----
            with tc.tile_pool(name="ph1", bufs=1) as p1, \
                 tc.tile_pool(name="ph1ps", bufs=2, space="PSUM") as ps1:
                xT_sb = p1.tile([128, EC, S], F32R)
                wT_sb = p1.tile([128, EC, 3 * FLOC], F32R)
                for c in range(EC):
                    nc.sync.dma_start(wT_sb[:, c, 0:2 * FLOC],
                                      wT[ts(c, 128), 0:2 * FLOC])
                    nc.sync.dma_start(xT_sb[:, c, 0:512], xT[ts(c, 128), 0:512])
                for tb in range(1, S // 512):
                    for c in range(EC):
                        nc.sync.dma_start(xT_sb[:, c, ts(tb, 512)],
                                          xT[ts(c, 128), ts(tb, 512)])
                for c in range(EC):
                    nc.sync.dma_start(wT_sb[:, c, 2 * FLOC:],
                                      wT[ts(c, 128), 2 * FLOC:])
                for c in range(OC):
                    nc.sync.dma_start(woT_sb[:, c, :], woT[ts(c, 128), :])
                nc.sync.dma_start(
                    v[:, :, :, 64:65],
                    vones.rearrange("p (t h one) -> p t h one", h=HLOC, one=1))

                # q/k first so kT/qT complete early and attention can begin
                # while the V projection still runs
                for tb in range(S // 512):
                    for ft in range(FT):
                        for gi, (dst, off) in enumerate(((qT, 0), (kT, FLOC))):
                            pq = ps1.tile([128, 512], F32, tag="pq")
                            lo = off + ft * 128
                            for c in range(EC):
                                nc.tensor.matmul(
                                    pq[:], wT_sb[:, c, lo:lo + 128],
                                    xT_sb[:, c, ts(tb, 512)],
                                    start=(c == 0), stop=(c == EC - 1))
                            nc.vector.tensor_scalar_add(
                                dst[:, ft, ts(tb, 512)], pq[:],
                                qkb_sb[:, gi * FT + ft:gi * FT + ft + 1])
                # V natural layout: [tok, vfeat] per 128-token tile
                for t in range(TT):
                    pv = ps1.tile([128, FLOC], F32, tag="pv")
                    for c in range(EC):
                        nc.tensor.matmul(
                            pv[:], xT_sb[:, c, ts(t, 128)],
                            wT_sb[:, c, 2 * FLOC:3 * FLOC],
                            start=(c == 0), stop=(c == EC - 1))
                    nc.vector.tensor_add(
                        v[:, t, :, 0:64],
                        pv.rearrange("p (h d) -> p h d", h=HLOC),
                        vbr_sb.rearrange("p (h d) -> p h d", h=HLOC))

            # ---- phase 2+3: attention with fused out_proj per query block ----
            with tc.tile_pool(name="ph2", bufs=16) as p2, \
                 tc.tile_pool(name="ph2oc", bufs=2) as p2oc, \
                 tc.tile_pool(name="ph3", bufs=2) as p3, \
                 tc.tile_pool(name="ph2ps", bufs=2, space="PSUM") as ps2, \
                 tc.tile_pool(name="ph2po", bufs=1, space="PSUM") as ps2o, \
                 tc.tile_pool(name="ph3ps", bufs=1, space="PSUM") as ps3:
                onum = 2
                for qb in range(QB):
                    for hp in range(HLOC // 2):
                        o0 = ps2o.tile([65, 512], F32, tag=f"o{onum % 3}")
                        o1 = ps2o.tile([65, 512], F32, tag=f"o{(onum + 1) % 3}")
                        onum += 2
                        oo = [o0, o1]
                        for kt in range(KT):
                            sps = ps2.tile([128, 2, 512], F32, tag="s")
                            ex = p2.tile([128, 2, 512], F32R, tag="exp")
                            for hh in range(2):
                                p0 = 64 * hh
                                nc.tensor.matmul(
                                    sps[:, hh, :],
                                    kT[p0:p0 + 64, hp, ts(kt, 128)],
                                    qT[p0:p0 + 64, hp, ts(qb, 512)],
                                    start=True, stop=True)
                            nc.scalar.activation(ex[:], sps[:], EXP, scale=scale)
                            for hh in range(2):
                                nc.tensor.matmul(
                                    oo[hh][:],
                                    v[:, kt, 2 * hp + hh, :],
                                    ex[:, hh, :],
                                    start=(kt == 0),
                                    stop=(kt == KT - 1))
                        for hh in range(2):
                            # copy out of PSUM promptly so the o slot frees for
                            # the next head pair; divide from the SBUF copy
                            rec = p2oc.tile([1, 512], F32, tag="rec")
                            nc.vector.reciprocal(rec[:], oo[hh][64:65, :])
                            oc = p2oc.tile([65, 512], F32, tag="oc")
                            nc.vector.tensor_copy(oc[0:64, :], oo[hh][0:64, :])
                            rep = p2oc.tile([64, 512], F32, tag="rep")
                            nc.gpsimd.partition_broadcast(rep[:], rec[:])
                            # outT chunk hp holds feats of heads (2hp, 2hp+1)
                            nc.vector.tensor_mul(
                                outT[64 * hh:64 * hh + 64, hp, ts(qb, 512)],
                                oc[0:64, :], rep[:])
                    # out_proj for this query block's token tiles
                    for t in range(TPQ * qb, TPQ * qb + TPQ):
                        fo = p3.tile([128, EOUT], F32, tag="fo")
                        for eb in range(EB):
                            w = min(512, EOUT - eb * 512)
                            po = ps3.tile([128, 512], F32, tag="po")
                            for c in range(OC):
                                nc.tensor.matmul(
                                    po[:, :w], outT[:, c, ts(t, 128)],
                                    woT_sb[:, c, eb * 512:eb * 512 + w],
                                    start=(c == 0), stop=(c == OC - 1))
                            nc.vector.tensor_copy(fo[:, eb * 512:eb * 512 + w],
                                                  po[:, :w])
                        nc.sync.dma_start(out[ts(t, 128), :], fo[:])


_CACHED = {}


def _get_module():
    if "nc" not in _CACHED:
        nc = bacc.Bacc("TRN2")
        build_mha(nc)
        nc.finalize()
        _CACHED["nc"] = nc
    return _CACHED["nc"]


def make_in_maps(query, w_in, b_in, w_o):
    """Host-side sharding: per-core input dicts (layout transforms included)."""
    E, HLoc, FLoc = D_MODEL, HL, FL
    woT_full = np.ascontiguousarray(w_o.T, dtype=np.float32)  # (e_in, e_out)
    vones_arr = np.ones((128, (SEQ // 128) * HLoc), np.float32)
    in_maps = []
    for core in range(N_CORES):
        b, g = divmod(core, GROUPS)
        rows = np.r_[g * FLoc:(g + 1) * FLoc,
                     E + g * FLoc:E + (g + 1) * FLoc,
                     2 * E + g * FLoc:2 * E + (g + 1) * FLoc]
        bl = b_in[rows].astype(np.float32)
        ft_n = FLoc // 128
        qkb_c = np.empty((128, 2 * ft_n), np.float32)
        for ft in range(ft_n):
            qkb_c[:, ft] = bl[ft * 128:(ft + 1) * 128]
            qkb_c[:, ft_n + ft] = bl[FLoc + ft * 128:FLoc + (ft + 1) * 128]
        vbr_c = np.ascontiguousarray(
            np.broadcast_to(bl[2 * FLoc:], (128, FLoc)))
        in_maps.append({
            "vones": vones_arr,
            "xT": np.ascontiguousarray(query[b].T, dtype=np.float32),
            "wT": np.ascontiguousarray(w_in[rows].T, dtype=np.float32),
            "qkb": qkb_c, "vbr": vbr_c,
            "woT": np.ascontiguousarray(woT_full[g * FLoc:(g + 1) * FLoc]),
        })
    return in_maps


def kernel(query, key, value, w_in, b_in, w_o, b_o, _trace=False):
    from concourse.bass_utils import run_bass_kernel_spmd
    query = np.asarray(query, dtype=np.float32)
    nc = _get_module()
    in_maps = make_in_maps(query, np.asarray(w_in), np.asarray(b_in),
                           np.asarray(w_o))
    res = run_bass_kernel_spmd(nc, in_maps, core_ids=list(range(N_CORES)),
                               trace=_trace)
    out = np.empty((N_BATCH, SEQ, D_MODEL), np.float32)
    for b in range(N_BATCH):
        acc = res.results[b * GROUPS]["out"].astype(np.float32)
        for g in range(1, GROUPS):
            acc = acc + res.results[b * GROUPS + g]["out"]
        out[b] = acc + np.asarray(b_o, dtype=np.float32)[None, :]
    if _trace:
        kernel.last_exec_time_ns = res.exec_time_ns
    return out
